# revision 1
# baseline (speedup 1.0000x reference)
"""APPNP GNN (MLP + K-hop propagation) as a multi-core Bass/Tile kernel for TRN2.

Algorithm (per hop): h <- (1-a) * Ahat @ h + a * h0, Ahat = D^-1/2 (A+I) D^-1/2.

Device strategy (8 cores, SPMD):
  - nodes row-partitioned: core c owns rows [c*R, (c+1)*R), R = nW*128
  - scaled state H' = dinv * h replicated in DRAM (H_full, Shared) on every core
  - per hop, per core: for each 128-row dst window, gather H'[src] rows for the
    window's incident edges (dma_gather, int16 idx -> sources bucketed into
    <=32768-row chunks), build per-128-edge one-hot S matrices on DVE
    (is_equal vs iota), segment-sum via PE matmuls accumulating in PSUM,
    then h_next = 0.9*dinv*agg + 0.1*h0, store H'_next = dinv*h_next,
    AllGather H_slice -> H_full.
  - edges are padded per (window, sub64, chunk) bucket to a multiple of 128,
    sized as the max across cores so the program is identical on all cores.
    Pad slots gather a dummy row (idx 0) and are killed by dstl_rel = -1.
"""

import math
import sys
from contextlib import ExitStack
from dataclasses import dataclass, field

import numpy as np

sys.path.insert(0, "/opt/trn_rl_repo")

import concourse.bacc as bacc
import concourse.bass as bass
import concourse.mybir as mybir
import concourse.tile as tile
from concourse._compat import cdiv

F32 = mybir.dt.float32
BF16 = mybir.dt.bfloat16
I16 = mybir.dt.int16
AF = mybir.ActivationFunctionType
ALU = mybir.AluOpType

WINDOW = 128
SUB = 64
NSUB = 2


@dataclass
class Cfg:
    N: int
    E: int          # edges before self loops
    F: int = 512
    H: int = 256
    C: int = 64
    K: int = 10
    alpha: float = 0.1
    n_cores: int = 8
    chunk_rows: int = 32768   # max rows addressable by int16 gather index
    G: int = 4                # windows per gather group
    mlp_block: int = 512      # rows per MLP block (<=512)
    unroll_hops: bool = False
    max_gather: int = 1024    # per-instruction idx limit (SWDGE ring capacity)
    n_queues: int = 4         # SWDGE queues to rotate gathers across
    parts: int = 1            # >1: partition AllGather into `parts` pieces,
                              # chunk == part, pass-per-part hop structure

    @property
    def R(self):  # rows per core, multiple of WINDOW
        return cdiv(cdiv(self.N, self.n_cores), WINDOW) * WINDOW

    @property
    def N_pad(self):
        return self.R * self.n_cores

    @property
    def nW(self):
        return self.R // WINDOW

    @property
    def chunk_bases(self):
        if self.parts > 1:
            out, b = [], 0
            for p in range(self.parts):
                out.append(b)
                b += self.n_cores * self.part_rows[p]
            return out
        return list(range(0, self.N_pad, self.chunk_rows))

    @property
    def chunk_sizes(self):
        if self.parts > 1:
            return [self.n_cores * r for r in self.part_rows]
        bs = self.chunk_bases
        return [min(self.chunk_rows, self.N_pad - b) for b in bs]

    @property
    def n_chunks(self):
        if self.parts > 1:
            return self.parts
        return len(self.chunk_bases)

    # -- parts>1 window partition: nW windows split into `parts` runs
    @property
    def part_wsizes(self):
        base = self.nW // self.parts
        rem = self.nW % self.parts
        return [base + (1 if p < rem else 0) for p in range(self.parts)]

    @property
    def part_woffs(self):
        out, o = [], 0
        for s in self.part_wsizes:
            out.append(o)
            o += s
        return out

    @property
    def part_rows(self):
        return [s * WINDOW for s in self.part_wsizes]

    @property
    def n_groups(self):
        return cdiv(self.nW, self.G)

    def group_windows(self, g):
        return range(g * self.G, min((g + 1) * self.G, self.nW))


@dataclass
class Plan:
    """Static (core-independent) program structure.

    One flat slot stream per core, ordered g -> chunk -> (window, sub within
    group), each bucket padded to a multiple of 128. Everything (gather idx
    wrapping, dstl columns, gather-buffer columns) is derived from this one
    layout.
    """
    tiles: np.ndarray            # [nW, NSUB, n_chunks] int, tiles per bucket
    ng: np.ndarray               # [n_groups, n_chunks] num_idxs per gather
    idx_col_off: np.ndarray      # [n_groups, n_chunks] column offset into idx dram
    gbuf_col_off: np.ndarray     # [nW, NSUB, n_chunks] tile-col offset within (g,c) gather buffer
    dstl_col_off: np.ndarray     # [nW, NSUB, n_chunks] tile-col offset into dstl tensor
    bucket_slot_off: np.ndarray  # [nW, NSUB, n_chunks] slot offset in the stream
    total_slots: int
    idx_cols_total: int
    dstl_cols_total: int
    gbuf_tiles_max: np.ndarray   # [n_chunks] max tile-cols of any group's gather buf


def make_plan(cfg: Cfg, counts_max: np.ndarray) -> Plan:
    """counts_max: [nW, NSUB, n_chunks] max-over-core edge counts per bucket."""
    padded = (np.ceil(counts_max / WINDOW).astype(np.int64)) * WINDOW
    tiles = padded // WINDOW

    ng = np.zeros((cfg.n_groups, cfg.n_chunks), dtype=np.int64)
    idx_col_off = np.zeros_like(ng)
    gbuf_col_off = np.zeros((cfg.nW, NSUB, cfg.n_chunks), dtype=np.int64)
    dstl_col_off = np.zeros_like(gbuf_col_off)
    bucket_slot_off = np.zeros_like(gbuf_col_off)

    off = 0
    if cfg.parts > 1:
        # stream order: part (outer) -> group -> (w, s)
        for c in range(cfg.n_chunks):
            for g in range(cfg.n_groups):
                idx_col_off[g, c] = off // 16
                seg0 = off
                for w in cfg.group_windows(g):
                    for s in range(NSUB):
                        bucket_slot_off[w, s, c] = off
                        gbuf_col_off[w, s, c] = (off - seg0) // WINDOW
                        dstl_col_off[w, s, c] = off // WINDOW
                        off += padded[w, s, c]
                ng[g, c] = off - seg0
    else:
        for g in range(cfg.n_groups):
            for c in range(cfg.n_chunks):
                idx_col_off[g, c] = off // 16
                seg0 = off
                for w in cfg.group_windows(g):
                    for s in range(NSUB):
                        bucket_slot_off[w, s, c] = off
                        gbuf_col_off[w, s, c] = (off - seg0) // WINDOW
                        dstl_col_off[w, s, c] = off // WINDOW
                        off += padded[w, s, c]
                ng[g, c] = off - seg0

    gmax = ng.max(axis=0) // WINDOW
    return Plan(tiles, ng, idx_col_off, gbuf_col_off, dstl_col_off,
                bucket_slot_off, off, off // 16, max(off // WINDOW, 1), gmax)


def host_prep(cfg: Cfg, x, W1, b1, W2, b2, edge_index):
    """Build per-core input maps + the static Plan."""
    N, R = cfg.N, cfg.R
    src = np.concatenate([edge_index[0], np.arange(N, dtype=np.int64)])
    dst = np.concatenate([edge_index[1], np.arange(N, dtype=np.int64)])
    src = src.astype(np.int64)
    dst = dst.astype(np.int64)

    deg = np.bincount(dst, minlength=N).astype(np.float64)
    dinv = (1.0 / np.sqrt(deg)).astype(np.float32)          # deg >= 1 (self loops)
    dinv_pad = np.ones(cfg.N_pad, dtype=np.float32)
    dinv_pad[:N] = dinv

    core_of = dst // R
    w_of = (dst % R) // WINDOW
    s_of = (dst % WINDOW) // SUB
    dstl_rel = (dst % SUB).astype(np.float32)
    if cfg.parts > 1:
        # H_full row layout: [part0: core0 rows.. core7 rows][part1: ...]...
        wpart = np.zeros(cfg.nW, dtype=np.int64)
        for p, (wo, ws) in enumerate(zip(cfg.part_woffs, cfg.part_wsizes)):
            wpart[wo:wo + ws] = p
        part_rows = np.array(cfg.part_rows, dtype=np.int64)
        part_woff_rows = np.array([o * WINDOW for o in cfg.part_woffs], dtype=np.int64)
        csrc = src // R
        lsrc = src % R
        psrc = wpart[lsrc // WINDOW]
        chunk_of = psrc
        idx_local = csrc * part_rows[psrc] + (lsrc - part_woff_rows[psrc])
    else:
        chunk_of = src // cfg.chunk_rows
        idx_local = (src - chunk_of * cfg.chunk_rows).astype(np.int64)

    nW, nC, nCh = cfg.nW, cfg.n_cores, cfg.n_chunks
    bucket = ((core_of * nW + w_of) * NSUB + s_of) * nCh + chunk_of
    n_buckets = nC * nW * NSUB * nCh
    counts = np.bincount(bucket, minlength=n_buckets).reshape(nC, nW, NSUB, nCh)
    counts_max = counts.max(axis=0)
    plan = make_plan(cfg, counts_max)

    bucket_slot_off = plan.bucket_slot_off
    total_slots = plan.total_slots

    # rank of each edge within its bucket
    order = np.argsort(bucket, kind="stable")
    sorted_bucket = bucket[order]
    seg_starts = np.searchsorted(sorted_bucket, np.arange(n_buckets))
    rank_sorted = np.arange(len(src)) - seg_starts[sorted_bucket]
    rank = np.empty_like(rank_sorted)
    rank[order] = rank_sorted

    slot_of = bucket_slot_off[w_of, s_of, chunk_of] + rank

    deg_sq = np.sqrt(deg).astype(np.float32)

    from ml_dtypes import bfloat16

    in_maps = []
    for c in range(nC):
        rows = slice(c * R, (c + 1) * R)
        xc = np.zeros((R, cfg.F), dtype=np.float32)
        take = min(N - c * R, R)
        xc[:take] = x[c * R : c * R + take]
        xT = np.ascontiguousarray(xc.T).astype(bfloat16)

        mask = core_of == c
        idx_stream = np.zeros(total_slots, dtype=np.int16)
        dstl_stream = np.full(total_slots, -1.0, dtype=np.float32)
        idx_stream[slot_of[mask]] = idx_local[mask].astype(np.int16)
        dstl_stream[slot_of[mask]] = dstl_rel[mask]

        # idx wrapped: [j%16, j//16], replicated to 128 partitions
        idx_w = idx_stream.reshape(-1, 16).T                 # [16, total/16]
        idx_rep = np.tile(idx_w, (8, 1)).astype(np.int16)    # [128, total/16]
        # dstl: [128, tiles] col t <-> edges [t*128,(t+1)*128), partition p = slot t*128+p
        dstl_cols = np.ascontiguousarray(
            dstl_stream.reshape(-1, WINDOW).T).astype(np.float32)  # [128, total/128]

        dv = dinv_pad[c * R : (c + 1) * R].reshape(nW, WINDOW).T  # [128, nW]
        rd = np.ones((R,), dtype=np.float32)
        rd[:take] = deg_sq[c * R : c * R + take]
        rd = rd.reshape(nW, WINDOW).T

        iota = np.tile(np.arange(SUB, dtype=np.float32), (WINDOW, 1))
        eye = np.eye(SUB, dtype=np.float32)

        in_maps.append({
            "xT": xT,
            "W1": W1.astype(bfloat16),
            "b1": b1.reshape(cfg.H, 1).astype(np.float32),
            "W2": W2.astype(bfloat16),
            "b2": b2.reshape(cfg.C, 1).astype(np.float32),
            "iota": iota.astype(bfloat16),
            "eye": eye,
            "idxs": np.ascontiguousarray(idx_rep),
            "dstl": dstl_cols,
            "dinv_col": np.ascontiguousarray(dv),
            "dinv09_col": np.ascontiguousarray((1.0 - cfg.alpha) * dv),
            "rdinv_col": np.ascontiguousarray(rd),
        })
    return in_maps, plan


def build_kernel(cfg: Cfg, plan: Plan):
    """Build the SPMD Bass program. Returns compiled nc."""
    nc = bacc.Bacc("TRN2", target_bir_lowering=False, debug=False,
                   num_devices=cfg.n_cores, num_swdge_queues=cfg.n_queues)
    _gq = [0]

    def emit_gather(gb_ap, src_ap, it_ap, ngc):
        """Split a stream gather into <=max_gather-idx instructions (SWDGE
        descriptor-ring capacity), rotating across SWDGE queues."""
        o = 0
        while o < ngc:
            n = min(cfg.max_gather, ngc - o)
            nc.gpsimd.dma_gather(
                gb_ap[:, o // 128:(o + n) // 128, :],
                src_ap,
                it_ap[:, o // 16:(o + n) // 16],
                n, n, cfg.C,
                queue_num=_gq[0] % cfg.n_queues)
            _gq[0] += 1
            o += n
    R, nW, C, H, F = cfg.R, cfg.nW, cfg.C, cfg.H, cfg.F

    xT_d = nc.dram_tensor("xT", [F, R], BF16, kind="ExternalInput")
    W1_d = nc.dram_tensor("W1", [F, H], BF16, kind="ExternalInput")
    b1_d = nc.dram_tensor("b1", [H, 1], F32, kind="ExternalInput")
    W2_d = nc.dram_tensor("W2", [H, C], BF16, kind="ExternalInput")
    b2_d = nc.dram_tensor("b2", [C, 1], F32, kind="ExternalInput")
    iota_d = nc.dram_tensor("iota", [WINDOW, SUB], BF16, kind="ExternalInput")
    eye_d = nc.dram_tensor("eye", [SUB, SUB], F32, kind="ExternalInput")
    idxs_d = nc.dram_tensor("idxs", [128, plan.idx_cols_total], I16, kind="ExternalInput")
    dstl_d = nc.dram_tensor("dstl", [128, plan.dstl_cols_total], F32, kind="ExternalInput")
    dinv_d = nc.dram_tensor("dinv_col", [WINDOW, nW], F32, kind="ExternalInput")
    dinv09_d = nc.dram_tensor("dinv09_col", [WINDOW, nW], F32, kind="ExternalInput")
    rdinv_d = nc.dram_tensor("rdinv_col", [WINDOW, nW], F32, kind="ExternalInput")
    out_d = nc.dram_tensor("out", [R, C], F32, kind="ExternalOutput")

    groups = [list(range(cfg.n_cores))]

    with tile.TileContext(nc) as tc, ExitStack() as st:
        # ---- persistent pools
        const = st.enter_context(tc.tile_pool(name="const", bufs=1))
        dram = st.enter_context(tc.tile_pool(name="dram", bufs=1, space="DRAM"))

        H_slice = dram.tile([R, C], F32)
        # AllGather sits at the TOP of the hop body: H_slice -> H_full, then
        # gathers read H_full. With For_i there is exactly one collective
        # instruction, satisfying the single-writer rule on Shared DRAM.
        n_hf = cfg.K if cfg.unroll_hops else 1
        if cfg.parts > 1:
            H_fulls = [[dram.tile([cfg.chunk_sizes[p], C], F32,
                                  addr_space="Shared",
                                  tag=f"hfull{i}_{p}", name=f"hfull{i}_{p}")
                        for p in range(cfg.parts)]
                       for i in range(n_hf)]
        else:
            H_fulls = [dram.tile([cfg.N_pad, C], F32, addr_space="Shared",
                                 tag=f"hfull{i}", name=f"hfull{i}")
                       for i in range(n_hf)]

        iota_sb = const.tile([WINDOW, SUB], BF16, tag="iota")
        nc.sync.dma_start(iota_sb[:], iota_d[:])
        eye_sb = const.tile([SUB, SUB], F32, tag="eye")
        nc.sync.dma_start(eye_sb[:], eye_d[:])
        dstl_sb = const.tile([128, plan.dstl_cols_total], F32, tag="dstl")
        nc.sync.dma_start(dstl_sb[:], dstl_d[:])
        dinv_sb = const.tile([WINDOW, nW], F32, tag="dinv")
        nc.sync.dma_start(dinv_sb[:], dinv_d[:])
        dinv09_sb = const.tile([WINDOW, nW], F32, tag="dinv09")
        nc.sync.dma_start(dinv09_sb[:], dinv09_d[:])
        rdinv_sb = const.tile([WINDOW, nW], F32, tag="rdinv")
        nc.sync.dma_start(rdinv_sb[:], rdinv_d[:])
        h0s_sb = const.tile([WINDOW, nW, C], F32, tag="h0s")  # 0.1 * h0, window-major

        W1t = []
        for kc in range(F // 128):
            t = const.tile([128, H], BF16, tag=f"w1_{kc}")
            nc.sync.dma_start(t[:], W1_d[kc * 128:(kc + 1) * 128, :])
            W1t.append(t)
        W2t = []
        for kc in range(H // 128):
            t = const.tile([128, C], BF16, tag=f"w2_{kc}")
            nc.sync.dma_start(t[:], W2_d[kc * 128:(kc + 1) * 128, :])
            W2t.append(t)
        b1c = []
        for hh in range(H // 128):
            t = const.tile([128, 1], F32, tag=f"b1_{hh}")
            nc.sync.dma_start(t[:], b1_d[hh * 128:(hh + 1) * 128, :])
            b1c.append(t)
        b2c = const.tile([C, 1], F32, tag="b2")
        nc.sync.dma_start(b2c[:], b2_d[:])

        # ---- phase 1: MLP -> h0s (SBUF) and H'_0 -> H_slice (DRAM)
        with tc.tile_pool(name="mlp", bufs=3) as mp, \
             tc.tile_pool(name="mlp_ps", bufs=2, space="PSUM") as pp1, \
             tc.tile_pool(name="mlp_ps2", bufs=2, space="PSUM") as pp2, \
             tc.tile_pool(name="mlp_pst", bufs=2, space="PSUM") as ppt:
            r0 = 0
            while r0 < R:
                B = min(cfg.mlp_block, R - r0)
                xt = []
                for kc in range(F // 128):
                    t = mp.tile([128, cfg.mlp_block], BF16, tag=f"x_{kc}")
                    nc.sync.dma_start(t[:, :B], xT_d[kc * 128:(kc + 1) * 128, r0:r0 + B])
                    xt.append(t)
                h1 = []
                for half in range(H // 128):
                    ps = pp1.tile([128, cfg.mlp_block], F32, tag=f"ps1_{half}")
                    for kc in range(F // 128):
                        nc.tensor.matmul(
                            ps[:, :B],
                            W1t[kc][:, half * 128:(half + 1) * 128],
                            xt[kc][:, :B],
                            start=(kc == 0), stop=(kc == F // 128 - 1))
                    h = mp.tile([128, cfg.mlp_block], BF16, tag=f"h1_{half}")
                    nc.scalar.activation(h[:, :B], ps[:, :B], AF.Relu, bias=b1c[half][:])
                    h1.append(h)
                ps2 = pp2.tile([C, cfg.mlp_block], F32, tag="ps2")
                for kc in range(H // 128):
                    nc.tensor.matmul(ps2[:, :B], W2t[kc][:], h1[kc][:, :B],
                                     start=(kc == 0), stop=(kc == H // 128 - 1))
                hT = mp.tile([C, cfg.mlp_block], F32, tag="hT")
                nc.scalar.activation(hT[:, :B], ps2[:, :B], AF.Identity, bias=b2c[:])
                for j in range(B // WINDOW):
                    w = (r0 // WINDOW) + j
                    pst = ppt.tile([WINDOW, C], F32, tag="pst")
                    nc.tensor.transpose(pst[:], hT[:, j * WINDOW:(j + 1) * WINDOW], eye_sb[:])
                    nc.vector.tensor_scalar_mul(h0s_sb[:, w, :], pst[:], cfg.alpha)
                    hp = mp.tile([WINDOW, C], F32, tag="hp")
                    nc.vector.tensor_scalar_mul(hp[:], pst[:], dinv_sb[:, w:w + 1])
                    nc.sync.dma_start(H_slice[w * WINDOW:(w + 1) * WINDOW, :], hp[:])
                r0 += B

        # ---- phase 2: K propagation hops
        hop_pools = {
            "idx": st.enter_context(tc.tile_pool(name="idx", bufs=3)),
            "gb": st.enter_context(tc.tile_pool(name="gb", bufs=4)),
            "gbc": st.enter_context(tc.tile_pool(name="gbc", bufs=2)),
            "S": st.enter_context(tc.tile_pool(name="S", bufs=8)),
            "hw": st.enter_context(tc.tile_pool(name="hw", bufs=4)),
            "ps": st.enter_context(tc.tile_pool(name="ps", bufs=4, space="PSUM")),
        }

        def hop_body(H_full, _iv=None):
            nc.gpsimd.collective_compute(
                "AllGather", ALU.bypass, replica_groups=groups,
                ins=[H_slice.opt()], outs=[H_full.opt()])
            gmax_cols = int(plan.ng.sum(axis=1).max() // 16)
            for g in range(cfg.n_groups):
                # one batched idx load per group (stream is contiguous g -> c)
                gsum = int(plan.ng[g, :].sum())
                icol0 = int(plan.idx_col_off[g, 0])
                it_g = hop_pools["idx"].tile([128, gmax_cols], I16, tag="idxg")
                nc.sync.dma_start(it_g[:, :gsum // 16],
                                  idxs_d[:, icol0:icol0 + gsum // 16])
                gbufs = {}
                for c in range(cfg.n_chunks):
                    ngc = int(plan.ng[g, c])
                    if ngc == 0:
                        continue
                    ioff = int(plan.idx_col_off[g, c]) - icol0
                    gb = hop_pools["gb"].tile(
                        [128, int(plan.gbuf_tiles_max[c]), C], F32, tag=f"gb{c}")
                    cb, cs = cfg.chunk_bases[c], cfg.chunk_sizes[c]
                    emit_gather(gb, H_full[cb:cb + cs, :],
                                it_g[:, ioff:ioff + ngc // 16], ngc)
                    gbc = hop_pools["gbc"].tile(
                        [128, int(plan.gbuf_tiles_max[c]), C], BF16, tag=f"gbc{c}")
                    nt = ngc // WINDOW
                    nc.scalar.activation(gbc[:, :nt, :], gb[:, :nt, :], AF.Identity)
                    gbufs[c] = gbc
                for w in cfg.group_windows(g):
                    ps = hop_pools["ps"].tile([WINDOW, C], F32, tag="agg")
                    for s in range(NSUB):
                        first = True
                        total_t = int(plan.tiles[w, s, :].sum())
                        done_t = 0
                        for c in range(cfg.n_chunks):
                            T = int(plan.tiles[w, s, c])
                            for t in range(T):
                                S = hop_pools["S"].tile([WINDOW, SUB], BF16, tag="S")
                                dcol = int(plan.dstl_col_off[w, s, c]) + t
                                nc.vector.tensor_scalar(
                                    S[:], iota_sb[:], dstl_sb[:, dcol:dcol + 1],
                                    None, op0=ALU.is_equal)
                                q = int(plan.gbuf_col_off[w, s, c]) + t
                                done_t += 1
                                nc.tensor.matmul(
                                    ps[s * SUB:(s + 1) * SUB, :],
                                    S[:], gbufs[c][:, q, :],
                                    start=first, stop=(done_t == total_t))
                                first = False
                        if first:
                            nc.vector.memset(ps[s * SUB:(s + 1) * SUB, :], 0.0)
                    hn = hop_pools["hw"].tile([WINDOW, C], F32, tag="hn")
                    nc.vector.scalar_tensor_tensor(
                        hn[:], ps[:], dinv09_sb[:, w:w + 1], h0s_sb[:, w, :],
                        op0=ALU.mult, op1=ALU.add)
                    hp = hop_pools["hw"].tile([WINDOW, C], F32, tag="hp2")
                    nc.vector.tensor_scalar_mul(hp[:], hn[:], dinv_sb[:, w:w + 1])
                    nc.sync.dma_start(H_slice[w * WINDOW:(w + 1) * WINDOW, :], hp[:])

        if cfg.parts > 1:
            acc_sb = const.tile([WINDOW, nW, C], F32, tag="acc")

        def hop_body_parts(HF):
            for p in range(cfg.parts):
                a = cfg.part_woffs[p] * WINDOW
                b = a + cfg.part_rows[p]
                nc.gpsimd.collective_compute(
                    "AllGather", ALU.bypass, replica_groups=groups,
                    ins=[H_slice[a:b, :].opt()], outs=[HF[p].opt()])
            for p in range(cfg.parts):
                last = p == cfg.parts - 1
                for g in range(cfg.n_groups):
                    ngc = int(plan.ng[g, p])
                    gb = None
                    if ngc > 0:
                        icol = int(plan.idx_col_off[g, p])
                        it = hop_pools["idx"].tile(
                            [128, int(plan.ng.max() // 16)], I16, tag="idx")
                        nc.sync.dma_start(it[:, :ngc // 16],
                                          idxs_d[:, icol:icol + ngc // 16])
                        gb = hop_pools["gb"].tile(
                            [128, int(plan.gbuf_tiles_max.max()), C], F32, tag="gb")
                        emit_gather(gb, HF[p][:], it, ngc)
                    for w in cfg.group_windows(g):
                        nT = int(plan.tiles[w, :, p].sum())
                        accw = acc_sb[:, w, :]
                        if nT == 0:
                            if p == 0:
                                nc.vector.memset(accw, 0.0)
                            if not last:
                                continue
                            ps = None
                        else:
                            ps = hop_pools["ps"].tile([WINDOW, C], F32, tag="agg")
                            for s in range(NSUB):
                                T = int(plan.tiles[w, s, p])
                                if T == 0:
                                    nc.vector.memset(ps[s * SUB:(s + 1) * SUB, :], 0.0)
                                    continue
                                for t in range(T):
                                    S = hop_pools["S"].tile([WINDOW, SUB], F32, tag="S")
                                    dcol = int(plan.dstl_col_off[w, s, p]) + t
                                    nc.vector.tensor_scalar(
                                        S[:], iota_sb[:], dstl_sb[:, dcol:dcol + 1],
                                        None, op0=ALU.is_equal)
                                    q = int(plan.gbuf_col_off[w, s, p]) + t
                                    nc.tensor.matmul(
                                        ps[s * SUB:(s + 1) * SUB, :],
                                        S[:], gb[:, q, :],
                                        start=(t == 0), stop=(t == T - 1))
                        if not last:
                            if ps is not None:
                                if p == 0:
                                    nc.vector.tensor_copy(accw, ps[:])
                                else:
                                    nc.vector.tensor_add(accw, accw, ps[:])
                            continue
                        # final part: combine and store H'
                        hn0 = hop_pools["hw"].tile([WINDOW, C], F32, tag="hn0")
                        if ps is not None:
                            nc.vector.tensor_add(hn0[:], accw, ps[:])
                        else:
                            nc.vector.tensor_copy(hn0[:], accw)
                        hn = hop_pools["hw"].tile([WINDOW, C], F32, tag="hn")
                        nc.vector.scalar_tensor_tensor(
                            hn[:], hn0[:], dinv09_sb[:, w:w + 1], h0s_sb[:, w, :],
                            op0=ALU.mult, op1=ALU.add)
                        hp = hop_pools["hw"].tile([WINDOW, C], F32, tag="hp2")
                        nc.vector.tensor_scalar_mul(hp[:], hn[:], dinv_sb[:, w:w + 1])
                        nc.sync.dma_start(H_slice[w * WINDOW:(w + 1) * WINDOW, :], hp[:])

        body = hop_body_parts if cfg.parts > 1 else hop_body
        if cfg.unroll_hops:
            for k in range(cfg.K):
                body(H_fulls[k])
        else:
            with tc.For_i(0, cfg.K, 1) as _i:
                body(H_fulls[0])

        # ---- phase 3: log_softmax
        with tc.tile_pool(name="sm", bufs=4) as smp, \
             tc.tile_pool(name="smc", bufs=4) as smc:
            for w in range(nW):
                hp = smp.tile([WINDOW, C], F32, tag="hp3")
                nc.sync.dma_start(hp[:], H_slice[w * WINDOW:(w + 1) * WINDOW, :])
                h = smp.tile([WINDOW, C], F32, tag="h3")
                nc.vector.tensor_scalar_mul(h[:], hp[:], rdinv_sb[:, w:w + 1])
                nm = smc.tile([WINDOW, 1], F32, tag="nm")
                nc.vector.tensor_reduce(nm[:], h[:], mybir.AxisListType.X,
                                        ALU.max, negate=True)
                e = smp.tile([WINDOW, C], F32, tag="e3")
                se = smc.tile([WINDOW, 1], F32, tag="se")
                nc.scalar.activation(e[:], h[:], AF.Exp, bias=nm[:], accum_out=se[:])
                ls = smc.tile([WINDOW, 1], F32, tag="ls")
                nc.scalar.activation(ls[:], se[:], AF.Ln)
                o = smp.tile([WINDOW, C], F32, tag="o3")
                nc.vector.tensor_scalar(o[:], h[:], nm[:], ls[:],
                                        op0=ALU.add, op1=ALU.subtract)
                nc.sync.dma_start(out_d[w * WINDOW:(w + 1) * WINDOW, :], o[:])

    nc.compile()
    return nc


def reference_np(cfg: Cfg, x, W1, b1, W2, b2, edge_index):
    h = np.maximum(x @ W1 + b1, 0.0)
    h = h @ W2 + b2
    N = cfg.N
    src = np.concatenate([edge_index[0], np.arange(N)]).astype(np.int64)
    dst = np.concatenate([edge_index[1], np.arange(N)]).astype(np.int64)
    deg = np.bincount(dst, minlength=N).astype(np.float64)
    dinv = 1.0 / np.sqrt(deg)
    norm = (dinv[src] * dinv[dst])[:, None].astype(np.float32)
    h0 = h
    for _ in range(cfg.K):
        msg = norm * h[src]
        agg = np.zeros_like(h)
        np.add.at(agg, dst, msg)
        h = (1 - cfg.alpha) * agg + cfg.alpha * h0
    m = h.max(axis=1, keepdims=True)
    ls = np.log(np.exp(h - m).sum(axis=1, keepdims=True))
    return h - m - ls

KERNEL_PARTS = 4

# test-harness knobs (not used by the grading path, which calls kernel() only)
PROFILE = False          # capture an NTFF neuron-profile on the next call
LAST_EXEC_NS = None      # max-core HW exec time of the last profiled call
LAST_TRACE = None        # trace dir of the last profiled call


# ---------------------------------------------------------------------------
# harness entry point
# ---------------------------------------------------------------------------
_BUILD_CACHE: dict = {}


def _get_compiled(cfg: Cfg, plan: Plan):
    key = (cfg.N, cfg.E, cfg.K, cfg.parts, cfg.G, cfg.unroll_hops,
           plan.tiles.tobytes())
    hit = _BUILD_CACHE.get(key)
    if hit is None:
        hit = build_kernel(cfg, plan)
        _BUILD_CACHE.clear()
        _BUILD_CACHE[key] = hit
    return hit


def kernel(x, W1, b1, W2, b2, edge_index):
    """Full (unsharded) inputs in, full [N, 64] log-softmax output out.

    Shards nodes/edges across the 8 NeuronCores internally (dst-partitioned
    windows + AllGather of the propagated state each hop).
    """
    from concourse.bass_utils import run_bass_kernel_spmd

    x = np.asarray(x, dtype=np.float32)
    W1 = np.asarray(W1, dtype=np.float32)
    b1 = np.asarray(b1, dtype=np.float32)
    W2 = np.asarray(W2, dtype=np.float32)
    b2 = np.asarray(b2, dtype=np.float32)
    edge_index = np.asarray(edge_index)

    N, F = x.shape
    H = W1.shape[1]
    C = W2.shape[1]
    E = edge_index.shape[1]
    cfg = Cfg(N=N, E=E, F=F, H=H, C=C, K=10, alpha=0.1, n_cores=8,
              G=2, unroll_hops=True, parts=KERNEL_PARTS)

    in_maps, plan = host_prep(cfg, x, W1, b1, W2, b2, edge_index)
    nc = _get_compiled(cfg, plan)
    res = run_bass_kernel_spmd(nc, in_maps, list(range(cfg.n_cores)),
                               trace=PROFILE)
    if PROFILE:
        global LAST_EXEC_NS, LAST_TRACE
        LAST_EXEC_NS = res.exec_time_ns
        LAST_TRACE = (res.instructions_and_trace or (None, None))[1]
    out = np.concatenate([res.results[i]["out"] for i in range(cfg.n_cores)],
                         axis=0)[:N]
    return np.ascontiguousarray(out, dtype=np.float32)



# revision 5
# speedup vs baseline: 1.5489x; 1.5489x over previous
"""APPNP GNN (MLP + K-hop propagation) as a multi-core Bass/Tile kernel for TRN2.

Algorithm (per hop): h <- (1-a) * Ahat @ h + a * h0, Ahat = D^-1/2 (A+I) D^-1/2.

Device strategy (8 cores, SPMD):
  - nodes row-partitioned: core c owns rows [c*R, (c+1)*R), R = nW*128
  - scaled state H' = dinv * h replicated in DRAM (H_full, Shared) on every core
  - per hop: one AllGather H_slice -> H_full, then for each 128-row dst window,
    gather H'[src] rows for the window's incident edges (dma_gather, int16 idx,
    sources bucketed into 4 chunks of 25088 rows), build the window's one-hot
    S tiles [128 edge-slots x 128 dst] in ONE DVE tensor_tensor is_equal
    (iota vs broadcast dstl), segment-sum via PE matmuls accumulating in PSUM,
    then h_next = 0.9*dinv*agg + 0.1*h0, store H'_next = dinv*h_next.
  - edges padded per (window, chunk) bucket to a multiple of 128, sized as the
    max across cores so the program is identical on all cores. Pad slots gather
    row 0 and are killed by dstl = -1 (S row all zero).
"""

import sys
from contextlib import ExitStack
from dataclasses import dataclass

import numpy as np

sys.path.insert(0, "/opt/trn_rl_repo")

import concourse.bacc as bacc
import concourse.bass as bass
import concourse.mybir as mybir
import concourse.tile as tile
from concourse._compat import cdiv

F32 = mybir.dt.float32
BF16 = mybir.dt.bfloat16
I16 = mybir.dt.int16
AF = mybir.ActivationFunctionType
ALU = mybir.AluOpType

WINDOW = 128


@dataclass
class Cfg:
    N: int
    E: int          # edges before self loops
    F: int = 512
    H: int = 256
    C: int = 64
    K: int = 10
    alpha: float = 0.1
    n_cores: int = 8
    chunk_rows: int = 25088   # N_pad / 4; <= 32768 for int16 gather idx
    G: int = 4                # windows per gather group
    mlp_block: int = 512      # rows per MLP block (<=512)
    max_gather: int = 1024    # per-instruction idx limit (SWDGE ring capacity)
    n_queues: int = 4         # SWDGE queues to rotate gathers across

    @property
    def R(self):  # rows per core, multiple of WINDOW
        return cdiv(cdiv(self.N, self.n_cores), WINDOW) * WINDOW

    @property
    def N_pad(self):
        return self.R * self.n_cores

    @property
    def nW(self):
        return self.R // WINDOW

    @property
    def n_chunks(self):
        return cdiv(self.N_pad, self.chunk_rows)

    @property
    def n_groups(self):
        return cdiv(self.nW, self.G)

    def group_windows(self, g):
        return range(g * self.G, min((g + 1) * self.G, self.nW))


@dataclass
class Plan:
    """Static (core-independent) program structure.

    Gather stream: g -> chunk -> (w within group), each (w, c) bucket padded
    to a multiple of 128 slots. S/dstl tiles ordered g -> w -> c -> t.
    """
    tiles: np.ndarray            # [nW, n_chunks] tiles per bucket
    ng: np.ndarray               # [n_groups, n_chunks] idxs per (g, c) stream
    idx_col_off: np.ndarray      # [n_groups, n_chunks] col offset into idx dram
    gbuf_col_off: np.ndarray     # [nW, n_chunks] tile col within (g,c) gather buf
    bucket_slot_off: np.ndarray  # [nW, n_chunks] slot offset in the stream
    w_tile_off: np.ndarray       # [nW] first dstl tile col of window w
    total_slots: int
    idx_cols_total: int
    dstl_tiles_total: int
    gbuf_tiles_max: np.ndarray   # [n_chunks] max tile count of any (g, c) buf
    T_max: int                   # max tiles of any window


def make_plan(cfg: Cfg, counts_max: np.ndarray) -> Plan:
    """counts_max: [nW, n_chunks] max-over-core edge counts per bucket."""
    padded = (np.ceil(counts_max / WINDOW).astype(np.int64)) * WINDOW
    tiles = padded // WINDOW

    ng = np.zeros((cfg.n_groups, cfg.n_chunks), dtype=np.int64)
    idx_col_off = np.zeros_like(ng)
    gbuf_col_off = np.zeros((cfg.nW, cfg.n_chunks), dtype=np.int64)
    bucket_slot_off = np.zeros_like(gbuf_col_off)

    off = 0
    for g in range(cfg.n_groups):
        for c in range(cfg.n_chunks):
            idx_col_off[g, c] = off // 16
            seg0 = off
            for w in cfg.group_windows(g):
                bucket_slot_off[w, c] = off
                gbuf_col_off[w, c] = (off - seg0) // WINDOW
                off += padded[w, c]
            ng[g, c] = off - seg0

    w_tile_off = np.zeros(cfg.nW, dtype=np.int64)
    t = 0
    for g in range(cfg.n_groups):
        for w in cfg.group_windows(g):
            w_tile_off[w] = t
            t += int(tiles[w, :].sum())

    gmax = np.zeros(cfg.n_chunks, dtype=np.int64)
    for c in range(cfg.n_chunks):
        for g in range(cfg.n_groups):
            s = sum(int(tiles[w, c]) for w in cfg.group_windows(g))
            gmax[c] = max(gmax[c], s)

    return Plan(tiles, ng, idx_col_off, gbuf_col_off, bucket_slot_off,
                w_tile_off, off, off // 16, t,
                gmax, int(tiles.sum(axis=1).max()))


def host_prep(cfg: Cfg, x, W1, b1, W2, b2, edge_index):
    """Build per-core input maps + the static Plan."""
    N, R = cfg.N, cfg.R
    src = np.concatenate([edge_index[0], np.arange(N, dtype=np.int64)]).astype(np.int64)
    dst = np.concatenate([edge_index[1], np.arange(N, dtype=np.int64)]).astype(np.int64)

    deg = np.bincount(dst, minlength=N).astype(np.float64)
    dinv = (1.0 / np.sqrt(deg)).astype(np.float32)          # deg >= 1 (self loops)
    dinv_pad = np.ones(cfg.N_pad, dtype=np.float32)
    dinv_pad[:N] = dinv

    core_of = dst // R
    w_of = (dst % R) // WINDOW
    dstl_rel = (dst % WINDOW).astype(np.float32)
    chunk_of = src // cfg.chunk_rows
    idx_local = (src - chunk_of * cfg.chunk_rows).astype(np.int64)

    nW, nC, nCh = cfg.nW, cfg.n_cores, cfg.n_chunks
    bucket = (core_of * nW + w_of) * nCh + chunk_of
    n_buckets = nC * nW * nCh
    counts = np.bincount(bucket, minlength=n_buckets).reshape(nC, nW, nCh)
    counts_max = counts.max(axis=0)
    plan = make_plan(cfg, counts_max)

    # rank of each edge within its bucket
    order = np.argsort(bucket, kind="stable")
    sorted_bucket = bucket[order]
    seg_starts = np.searchsorted(sorted_bucket, np.arange(n_buckets))
    rank_sorted = np.arange(len(src)) - seg_starts[sorted_bucket]
    rank = np.empty_like(rank_sorted)
    rank[order] = rank_sorted

    slot_of = plan.bucket_slot_off[w_of, chunk_of] + rank

    deg_sq = np.sqrt(deg).astype(np.float32)

    from ml_dtypes import bfloat16

    # dstl tile layout: window-major tile runs (w_tile_off), c-major within w
    def dstl_cols_for(mask, slots):
        """Return [128, dstl_tiles_total] bf16 dstl columns."""
        out = np.full((plan.dstl_tiles_total, WINDOW), -1.0, dtype=np.float32)
        # map stream slot -> (w, c, t, p) -> dstl tile col
        sl = slots[mask]
        w = w_of[mask]
        c = chunk_of[mask]
        rel = sl - plan.bucket_slot_off[w, c]
        t_in_bucket = rel // WINDOW
        p = rel % WINDOW
        # tile col = w_tile_off[w] + (tiles of previous chunks of w) + t
        prev = np.zeros((nW, nCh), dtype=np.int64)
        cum = np.cumsum(plan.tiles, axis=1)
        prev[:, 1:] = cum[:, :-1]
        col = plan.w_tile_off[w] + prev[w, c] + t_in_bucket
        out[col, p] = dstl_rel[mask]
        return np.ascontiguousarray(out.T).astype(bfloat16)  # [128, tiles]

    in_maps = []
    for core in range(nC):
        rows = slice(core * R, (core + 1) * R)
        xc = np.zeros((R, cfg.F), dtype=np.float32)
        take = min(N - core * R, R)
        xc[:take] = x[core * R: core * R + take]
        xT = np.ascontiguousarray(xc.T).astype(bfloat16)

        mask = core_of == core
        idx_stream = np.zeros(plan.total_slots, dtype=np.int16)
        idx_stream[slot_of[mask]] = idx_local[mask].astype(np.int16)
        # idx wrapped: [j%16, j//16], replicated to 128 partitions
        idx_w = idx_stream.reshape(-1, 16).T                 # [16, total/16]
        idx_rep = np.tile(idx_w, (8, 1)).astype(np.int16)    # [128, total/16]

        dstl_cols = dstl_cols_for(mask, slot_of)

        dv = dinv_pad[core * R: (core + 1) * R].reshape(nW, WINDOW).T  # [128, nW]
        rd = np.ones((R,), dtype=np.float32)
        rd[:take] = deg_sq[core * R: core * R + take]
        rd = rd.reshape(nW, WINDOW).T

        iota = np.tile(np.arange(WINDOW, dtype=np.float32), (WINDOW, 1))
        iota_wide = np.tile(iota[:, None, :], (1, plan.T_max, 1))
        eye = np.eye(64, dtype=np.float32)

        in_maps.append({
            "xT": xT,
            "W1": W1.astype(bfloat16),
            "b1": b1.reshape(cfg.H, 1).astype(np.float32),
            "W2": W2.astype(bfloat16),
            "b2": b2.reshape(cfg.C, 1).astype(np.float32),
            "iota_wide": iota_wide.astype(bfloat16),
            "eye": eye,
            "idxs": np.ascontiguousarray(idx_rep),
            "dstl": dstl_cols,
            "dinv_col": np.ascontiguousarray(dv),
            "dinv09_col": np.ascontiguousarray((1.0 - cfg.alpha) * dv),
            "rdinv_col": np.ascontiguousarray(rd),
        })
    return in_maps, plan


def build_kernel(cfg: Cfg, plan: Plan):
    """Build the SPMD Bass program. Returns compiled nc."""
    nc = bacc.Bacc("TRN2", target_bir_lowering=False, debug=False,
                   num_devices=cfg.n_cores, num_swdge_queues=cfg.n_queues)
    _gq = [0]

    def emit_gather(gb_ap, src_ap, it_ap, ngc):
        o = 0
        while o < ngc:
            n = min(cfg.max_gather, ngc - o)
            nc.gpsimd.dma_gather(
                gb_ap[:, o // 128:(o + n) // 128, :],
                src_ap,
                it_ap[:, o // 16:(o + n) // 16],
                n, n, cfg.C,
                queue_num=_gq[0] % cfg.n_queues)
            _gq[0] += 1
            o += n

    R, nW, C, H, F = cfg.R, cfg.nW, cfg.C, cfg.H, cfg.F
    nCh = cfg.n_chunks

    xT_d = nc.dram_tensor("xT", [F, R], BF16, kind="ExternalInput")
    W1_d = nc.dram_tensor("W1", [F, H], BF16, kind="ExternalInput")
    b1_d = nc.dram_tensor("b1", [H, 1], F32, kind="ExternalInput")
    W2_d = nc.dram_tensor("W2", [H, C], BF16, kind="ExternalInput")
    b2_d = nc.dram_tensor("b2", [C, 1], F32, kind="ExternalInput")
    iota_d = nc.dram_tensor("iota_wide", [WINDOW, plan.T_max, WINDOW], BF16,
                            kind="ExternalInput")
    eye_d = nc.dram_tensor("eye", [64, 64], F32, kind="ExternalInput")
    idxs_d = nc.dram_tensor("idxs", [128, plan.idx_cols_total], I16,
                            kind="ExternalInput")
    dstl_d = nc.dram_tensor("dstl", [128, plan.dstl_tiles_total], BF16,
                            kind="ExternalInput")
    dinv_d = nc.dram_tensor("dinv_col", [WINDOW, nW], F32, kind="ExternalInput")
    dinv09_d = nc.dram_tensor("dinv09_col", [WINDOW, nW], F32, kind="ExternalInput")
    rdinv_d = nc.dram_tensor("rdinv_col", [WINDOW, nW], F32, kind="ExternalInput")
    out_d = nc.dram_tensor("out", [R, C], F32, kind="ExternalOutput")

    groups = [list(range(cfg.n_cores))]

    with tile.TileContext(nc) as tc, ExitStack() as st:
        const = st.enter_context(tc.tile_pool(name="const", bufs=1))
        dram = st.enter_context(tc.tile_pool(name="dram", bufs=1, space="DRAM"))

        H_slice = dram.tile([R, C], F32)
        H_fulls = [dram.tile([cfg.N_pad, C], F32, addr_space="Shared",
                             tag=f"hfull{i}", name=f"hfull{i}")
                   for i in range(cfg.K)]

        iota_sb = const.tile([WINDOW, plan.T_max, WINDOW], BF16, tag="iota")
        nc.sync.dma_start(iota_sb[:], iota_d[:])
        eye_sb = const.tile([64, 64], F32, tag="eye")
        nc.sync.dma_start(eye_sb[:], eye_d[:])
        dstl_sb = const.tile([128, plan.dstl_tiles_total], BF16, tag="dstl")
        nc.sync.dma_start(dstl_sb[:], dstl_d[:])
        dinv_sb = const.tile([WINDOW, nW], F32, tag="dinv")
        nc.sync.dma_start(dinv_sb[:], dinv_d[:])
        dinv09_sb = const.tile([WINDOW, nW], F32, tag="dinv09")
        nc.sync.dma_start(dinv09_sb[:], dinv09_d[:])
        rdinv_sb = const.tile([WINDOW, nW], F32, tag="rdinv")
        nc.sync.dma_start(rdinv_sb[:], rdinv_d[:])
        h0s_sb = const.tile([WINDOW, nW, C], F32, tag="h0s")  # 0.1 * h0

        W1t = []
        for kc in range(F // 128):
            t = const.tile([128, H], BF16, tag=f"w1_{kc}")
            nc.sync.dma_start(t[:], W1_d[kc * 128:(kc + 1) * 128, :])
            W1t.append(t)
        W2t = []
        for kc in range(H // 128):
            t = const.tile([128, C], BF16, tag=f"w2_{kc}")
            nc.sync.dma_start(t[:], W2_d[kc * 128:(kc + 1) * 128, :])
            W2t.append(t)
        b1c = []
        for hh in range(H // 128):
            t = const.tile([128, 1], F32, tag=f"b1_{hh}")
            nc.sync.dma_start(t[:], b1_d[hh * 128:(hh + 1) * 128, :])
            b1c.append(t)
        b2c = const.tile([C, 1], F32, tag="b2")
        nc.sync.dma_start(b2c[:], b2_d[:])

        # ---- phase 1: MLP -> h0s (SBUF) and H'_0 -> H_slice (DRAM)
        with tc.tile_pool(name="mlp", bufs=3) as mp, \
             tc.tile_pool(name="mlp_ps", bufs=2, space="PSUM") as pp1, \
             tc.tile_pool(name="mlp_ps2", bufs=2, space="PSUM") as pp2, \
             tc.tile_pool(name="mlp_pst", bufs=2, space="PSUM") as ppt:
            r0 = 0
            while r0 < R:
                B = min(cfg.mlp_block, R - r0)
                xt = []
                for kc in range(F // 128):
                    t = mp.tile([128, cfg.mlp_block], BF16, tag=f"x_{kc}")
                    nc.sync.dma_start(t[:, :B], xT_d[kc * 128:(kc + 1) * 128, r0:r0 + B])
                    xt.append(t)
                h1 = []
                for half in range(H // 128):
                    ps = pp1.tile([128, cfg.mlp_block], F32, tag=f"ps1_{half}")
                    for kc in range(F // 128):
                        nc.tensor.matmul(
                            ps[:, :B],
                            W1t[kc][:, half * 128:(half + 1) * 128],
                            xt[kc][:, :B],
                            start=(kc == 0), stop=(kc == F // 128 - 1))
                    h = mp.tile([128, cfg.mlp_block], BF16, tag=f"h1_{half}")
                    nc.scalar.activation(h[:, :B], ps[:, :B], AF.Relu, bias=b1c[half][:])
                    h1.append(h)
                ps2 = pp2.tile([C, cfg.mlp_block], F32, tag="ps2")
                for kc in range(H // 128):
                    nc.tensor.matmul(ps2[:, :B], W2t[kc][:], h1[kc][:, :B],
                                     start=(kc == 0), stop=(kc == H // 128 - 1))
                hT = mp.tile([C, cfg.mlp_block], F32, tag="hT")
                nc.scalar.activation(hT[:, :B], ps2[:, :B], AF.Identity, bias=b2c[:])
                for j in range(B // WINDOW):
                    w = (r0 // WINDOW) + j
                    pst = ppt.tile([WINDOW, C], F32, tag="pst")
                    nc.tensor.transpose(pst[:], hT[:, j * WINDOW:(j + 1) * WINDOW], eye_sb[:])
                    nc.vector.tensor_scalar_mul(h0s_sb[:, w, :], pst[:], cfg.alpha)
                    hp = mp.tile([WINDOW, C], F32, tag="hp")
                    nc.vector.tensor_scalar_mul(hp[:], pst[:], dinv_sb[:, w:w + 1])
                    nc.sync.dma_start(H_slice[w * WINDOW:(w + 1) * WINDOW, :], hp[:])
                r0 += B

        # ---- phase 2: K propagation hops
        hop_pools = {
            "idx": st.enter_context(tc.tile_pool(name="idx", bufs=3)),
            "gb": st.enter_context(tc.tile_pool(name="gb", bufs=2)),
            "gbc": st.enter_context(tc.tile_pool(name="gbc", bufs=3)),
            "S": st.enter_context(tc.tile_pool(name="S", bufs=2)),
            "hw": st.enter_context(tc.tile_pool(name="hw", bufs=4)),
            "ps": st.enter_context(tc.tile_pool(name="ps", bufs=4, space="PSUM")),
        }

        def hop_body(H_full):
            nc.gpsimd.collective_compute(
                "AllGather", ALU.bypass, replica_groups=groups,
                ins=[H_slice.opt()], outs=[H_full.opt()])
            gmax_cols = int(plan.ng.sum(axis=1).max() // 16)
            for g in range(cfg.n_groups):
                gsum = int(plan.ng[g, :].sum())
                icol0 = int(plan.idx_col_off[g, 0])
                it_g = hop_pools["idx"].tile([128, gmax_cols], I16, tag="idxg")
                nc.sync.dma_start(it_g[:, :gsum // 16],
                                  idxs_d[:, icol0:icol0 + gsum // 16])
                gbufs = {}
                for c in range(nCh):
                    ngc = int(plan.ng[g, c])
                    if ngc == 0:
                        continue
                    ioff = int(plan.idx_col_off[g, c]) - icol0
                    gb = hop_pools["gb"].tile(
                        [128, int(plan.gbuf_tiles_max[c]), C], F32, tag=f"gb{c}")
                    cb = c * cfg.chunk_rows
                    cs = min(cfg.chunk_rows, cfg.N_pad - cb)
                    emit_gather(gb, H_full[cb:cb + cs, :],
                                it_g[:, ioff:ioff + ngc // 16], ngc)
                    gbufs[c] = gb
                for w in cfg.group_windows(g):
                    T_w = int(plan.tiles[w, :].sum())
                    if T_w > 0:
                        S = hop_pools["S"].tile([128, plan.T_max, 128], BF16, tag="S")
                        d0 = int(plan.w_tile_off[w])
                        bc = dstl_sb[:, d0:d0 + T_w, None].broadcast_to(
                            (128, T_w, 128))
                        nc.vector.tensor_tensor(S[:, :T_w, :], iota_sb[:, :T_w, :],
                                                bc, op=ALU.is_equal)
                        gbc = hop_pools["gbc"].tile(
                            [128, plan.T_max, C], BF16, tag="gbc")
                        done = 0
                        for c in range(nCh):
                            T = int(plan.tiles[w, c])
                            if T == 0:
                                continue
                            q = int(plan.gbuf_col_off[w, c])
                            nc.scalar.activation(gbc[:, done:done + T, :],
                                                 gbufs[c][:, q:q + T, :],
                                                 AF.Identity)
                            done += T
                    ps = hop_pools["ps"].tile([WINDOW, C], F32, tag="agg")
                    for t in range(T_w):
                        nc.tensor.matmul(
                            ps[:],
                            S[:, t, :], gbc[:, t, :],
                            start=(t == 0), stop=(t == T_w - 1))
                    if T_w == 0:
                        nc.vector.memset(ps[:], 0.0)
                    hn = hop_pools["hw"].tile([WINDOW, C], F32, tag="hn")
                    nc.vector.scalar_tensor_tensor(
                        hn[:], ps[:], dinv09_sb[:, w:w + 1], h0s_sb[:, w, :],
                        op0=ALU.mult, op1=ALU.add)
                    hp = hop_pools["hw"].tile([WINDOW, C], F32, tag="hp2")
                    nc.vector.tensor_scalar_mul(hp[:], hn[:], dinv_sb[:, w:w + 1])
                    nc.sync.dma_start(H_slice[w * WINDOW:(w + 1) * WINDOW, :], hp[:])

        for k in range(cfg.K):
            hop_body(H_fulls[k])

        # ---- phase 3: log_softmax
        with tc.tile_pool(name="sm", bufs=4) as smp, \
             tc.tile_pool(name="smc", bufs=4) as smc:
            for w in range(nW):
                hp = smp.tile([WINDOW, C], F32, tag="hp3")
                nc.sync.dma_start(hp[:], H_slice[w * WINDOW:(w + 1) * WINDOW, :])
                h = smp.tile([WINDOW, C], F32, tag="h3")
                nc.vector.tensor_scalar_mul(h[:], hp[:], rdinv_sb[:, w:w + 1])
                nm = smc.tile([WINDOW, 1], F32, tag="nm")
                nc.vector.tensor_reduce(nm[:], h[:], mybir.AxisListType.X,
                                        ALU.max, negate=True)
                e = smp.tile([WINDOW, C], F32, tag="e3")
                se = smc.tile([WINDOW, 1], F32, tag="se")
                nc.scalar.activation(e[:], h[:], AF.Exp, bias=nm[:], accum_out=se[:])
                ls = smc.tile([WINDOW, 1], F32, tag="ls")
                nc.scalar.activation(ls[:], se[:], AF.Ln)
                o = smp.tile([WINDOW, C], F32, tag="o3")
                nc.vector.tensor_scalar(o[:], h[:], nm[:], ls[:],
                                        op0=ALU.add, op1=ALU.subtract)
                nc.sync.dma_start(out_d[w * WINDOW:(w + 1) * WINDOW, :], o[:])

    nc.compile()
    return nc


# test-harness knobs (not used by the grading path, which calls kernel() only)
PROFILE = False
LAST_EXEC_NS = None
LAST_TRACE = None

_BUILD_CACHE: dict = {}


def _get_compiled(cfg: Cfg, plan: Plan):
    key = (cfg.N, cfg.E, cfg.K, cfg.G, plan.tiles.tobytes())
    hit = _BUILD_CACHE.get(key)
    if hit is None:
        hit = build_kernel(cfg, plan)
        _BUILD_CACHE.clear()
        _BUILD_CACHE[key] = hit
    return hit


def kernel(x, W1, b1, W2, b2, edge_index):
    """Full (unsharded) inputs in, full [N, 64] log-softmax output out."""
    from concourse.bass_utils import run_bass_kernel_spmd

    x = np.asarray(x, dtype=np.float32)
    W1 = np.asarray(W1, dtype=np.float32)
    b1 = np.asarray(b1, dtype=np.float32)
    W2 = np.asarray(W2, dtype=np.float32)
    b2 = np.asarray(b2, dtype=np.float32)
    edge_index = np.asarray(edge_index)

    N, F = x.shape
    H = W1.shape[1]
    C = W2.shape[1]
    E = edge_index.shape[1]
    cfg = Cfg(N=N, E=E, F=F, H=H, C=C, K=10, alpha=0.1, n_cores=8)

    in_maps, plan = host_prep(cfg, x, W1, b1, W2, b2, edge_index)
    nc = _get_compiled(cfg, plan)
    res = run_bass_kernel_spmd(nc, in_maps, list(range(cfg.n_cores)),
                               trace=PROFILE)
    if PROFILE:
        global LAST_EXEC_NS, LAST_TRACE
        LAST_EXEC_NS = res.exec_time_ns
        LAST_TRACE = (res.instructions_and_trace or (None, None))[1]
    out = np.concatenate([res.results[i]["out"] for i in range(cfg.n_cores)],
                         axis=0)[:N]
    return np.ascontiguousarray(out, dtype=np.float32)


# revision 6
# speedup vs baseline: 1.6468x; 1.0632x over previous
"""APPNP GNN (MLP + K-hop propagation) as a multi-core Bass/Tile kernel for TRN2.

Algorithm (per hop): h <- (1-a) * Ahat @ h + a * h0, Ahat = D^-1/2 (A+I) D^-1/2.

Device strategy (8 cores, SPMD):
  - nodes row-partitioned: core c owns rows [c*R, (c+1)*R), R = nW*128
  - scaled state H' = dinv * h kept in DRAM as bf16 rows padded to 256B
    ([*, 128] bf16, first 64 cols live), replicated via AllGather
  - the per-core slice is split in 4 quarters; each hop runs 4 quarter
    AllGathers so gathers on chunk c start as soon as quarter c arrived
  - per hop, per 128-row dst window: gather H'[src] rows for the window's
    incident edges (dma_gather with 128-byte elems at 256B stride, int16 idx
    bucketed into the 4 chunks), build the window's one-hot S tiles
    [128 edge-slots x 128 dst] in ONE DVE tensor_tensor is_equal
    (iota vs broadcast dstl), segment-sum via PE matmuls accumulating in
    PSUM; the teleport term alpha*h0 enters the PSUM chain as an
    identity-stationary matmul, so the epilogue is a single DVE multiply:
    H'_next = 0.9*dinv^2*(agg + h0pre), h0pre = (alpha/0.9)*sqrt(deg)*h0.
  - edges padded per (window, chunk) bucket to a multiple of 128, sized as the
    max across cores so the program is identical on all cores. Pad slots gather
    row 0 and are killed by dstl = -1 (S row all zero).
"""

import sys
from contextlib import ExitStack
from dataclasses import dataclass

import numpy as np

sys.path.insert(0, "/opt/trn_rl_repo")

import concourse.bacc as bacc
import concourse.bass as bass
import concourse.mybir as mybir
import concourse.tile as tile
from concourse import ap_utils
from concourse.bass import MemorySpace
from concourse._compat import cdiv, exact_div

F32 = mybir.dt.float32
BF16 = mybir.dt.bfloat16
I16 = mybir.dt.int16
AF = mybir.ActivationFunctionType
ALU = mybir.AluOpType

WINDOW = 128
ROWB = 128          # bf16 row stride in elements (256B); first 64 live


def round_up(x, m):
    return (x + m - 1) // m * m


def dma_gather128(gp, out_ap, in_ap, idxs_ap, num_idxs, num_idxs_reg,
                  elem_size, elem_step, queue_num=0, single_packet=True):
    """bass.BassGpSimd.dma_gather with the elem%256B assert relaxed to 128B
    (row stride must still be a 256B multiple)."""
    self = gp
    self._assert_queue_num(queue_num)
    assert idxs_ap.dtype == mybir.dt.int16
    assert in_ap.space == MemorySpace.DRAM
    assert in_ap.dtype == out_ap.dtype
    elem_size_bytes = elem_size * mybir.dt.size(in_ap.dtype)
    assert elem_size_bytes > 0 and elem_size_bytes % 128 == 0
    assert idxs_ap.space == MemorySpace.SBUF
    assert out_ap.space == MemorySpace.SBUF
    assert ap_utils.ap_is_contiguous(in_ap.ap[1:])
    assert ap_utils.ap_is_contiguous(out_ap.ap[1:])
    assert ap_utils.ap_is_contiguous(idxs_ap.ap[1:])
    assert in_ap.ap[-1][1] == out_ap.ap[-1][1] == elem_size
    assert out_ap.ap[0][1] * out_ap.ap[1][1] == round_up(num_idxs, 128)
    assert in_ap.ap[0][0] == elem_step
    stride_bytes = elem_step * mybir.dt.size(in_ap.dtype)
    stride_bytes_256 = exact_div(stride_bytes, 256)
    assert stride_bytes_256 < 256

    _in_ap = self.lower_ap_dma(in_ap, for_custom_bir_dma=True)
    _idxs_ap = self.lower_ap(idxs_ap)
    _out_ap = self.lower_ap(out_ap)
    return self.add_instruction(
        mybir.InstDMAGatherAnt(
            name=self.bass.get_next_instruction_name(),
            ins=[*_in_ap, _idxs_ap,
                 self.lower_val_access(self.to_reg(num_idxs_reg))],
            outs=[_out_ap],
            transpose=False,
            num_idxs=num_idxs,
            elem_size=elem_size,
            stride_bytes_256=stride_bytes_256,
            gen_mode=0,
            single_packet=single_packet,
            queue_num=queue_num,
            sbuf_tokens_per_rank=0,
            sbuf_free_dim_per_rank=0,
            sbuf_free_dim_pad_per_rank=0,
            sbuf_byte_offset=0,
        ))


@dataclass
class Cfg:
    N: int
    E: int          # edges before self loops
    F: int = 512
    H: int = 256
    C: int = 64
    K: int = 10
    alpha: float = 0.1
    n_cores: int = 8
    n_parts: int = 4          # slice quarters == src chunks
    G: int = 4                # windows per gather group
    mlp_block: int = 512      # rows per MLP block (<=512)
    max_gather: int = 1024    # per-instruction idx limit (SWDGE ring capacity)
    n_queues: int = 4         # SWDGE queues to rotate gathers across

    @property
    def R(self):  # rows per core: multiple of 128 * n_parts
        q = WINDOW * self.n_parts
        return cdiv(cdiv(self.N, self.n_cores), q) * q

    @property
    def part_rows(self):
        return self.R // self.n_parts

    @property
    def chunk_rows(self):     # rows per AllGather'd chunk (all cores' part p)
        return self.part_rows * self.n_cores

    @property
    def N_pad(self):
        return self.R * self.n_cores

    @property
    def nW(self):
        return self.R // WINDOW

    @property
    def n_chunks(self):
        return self.n_parts

    @property
    def n_groups(self):
        return cdiv(self.nW, self.G)

    def group_windows(self, g):
        return range(g * self.G, min((g + 1) * self.G, self.nW))


@dataclass
class Plan:
    tiles: np.ndarray            # [nW, n_chunks] tiles per bucket
    ng: np.ndarray               # [n_groups, n_chunks] idxs per (g, c) stream
    idx_col_off: np.ndarray      # [n_groups, n_chunks] col offset into idx dram
    gbuf_col_off: np.ndarray     # [nW, n_chunks] tile col within (g,c) gather buf
    bucket_slot_off: np.ndarray  # [nW, n_chunks] slot offset in the stream
    w_tile_off: np.ndarray       # [nW] first dstl tile col of window w
    total_slots: int
    idx_cols_total: int
    dstl_tiles_total: int
    gbuf_tiles_max: np.ndarray   # [n_chunks] max tile count of any (g, c) buf
    T_max: int                   # max tiles of any window


def make_plan(cfg: Cfg, counts_max: np.ndarray) -> Plan:
    padded = (np.ceil(counts_max / WINDOW).astype(np.int64)) * WINDOW
    tiles = padded // WINDOW

    ng = np.zeros((cfg.n_groups, cfg.n_chunks), dtype=np.int64)
    idx_col_off = np.zeros_like(ng)
    gbuf_col_off = np.zeros((cfg.nW, cfg.n_chunks), dtype=np.int64)
    bucket_slot_off = np.zeros_like(gbuf_col_off)

    off = 0
    for g in range(cfg.n_groups):
        for c in range(cfg.n_chunks):
            idx_col_off[g, c] = off // 16
            seg0 = off
            for w in cfg.group_windows(g):
                bucket_slot_off[w, c] = off
                gbuf_col_off[w, c] = (off - seg0) // WINDOW
                off += padded[w, c]
            ng[g, c] = off - seg0

    w_tile_off = np.zeros(cfg.nW, dtype=np.int64)
    t = 0
    for g in range(cfg.n_groups):
        for w in cfg.group_windows(g):
            w_tile_off[w] = t
            t += int(tiles[w, :].sum())

    gmax = np.zeros(cfg.n_chunks, dtype=np.int64)
    for c in range(cfg.n_chunks):
        for g in range(cfg.n_groups):
            s = sum(int(tiles[w, c]) for w in cfg.group_windows(g))
            gmax[c] = max(gmax[c], s)

    return Plan(tiles, ng, idx_col_off, gbuf_col_off, bucket_slot_off,
                w_tile_off, off, off // 16, t,
                gmax, int(tiles.sum(axis=1).max()))


def host_prep(cfg: Cfg, x, W1, b1, W2, b2, edge_index):
    N, R, PR = cfg.N, cfg.R, cfg.part_rows
    src = np.concatenate([edge_index[0], np.arange(N, dtype=np.int64)]).astype(np.int64)
    dst = np.concatenate([edge_index[1], np.arange(N, dtype=np.int64)]).astype(np.int64)

    deg = np.bincount(dst, minlength=N).astype(np.float64)
    dinv = (1.0 / np.sqrt(deg)).astype(np.float32)
    dinv_pad = np.ones(cfg.N_pad, dtype=np.float32)
    dinv_pad[:N] = dinv

    core_of = dst // R
    w_of = (dst % R) // WINDOW
    dstl_rel = (dst % WINDOW).astype(np.float32)
    # chunk p = union over cores of each core's slice quarter p;
    # AllGather_p output position: src_core * PR + (src % R) % PR
    src_off = src % R
    chunk_of = src_off // PR
    idx_local = (src // R) * PR + (src_off % PR)

    nW, nC, nCh = cfg.nW, cfg.n_cores, cfg.n_chunks
    bucket = (core_of * nW + w_of) * nCh + chunk_of
    n_buckets = nC * nW * nCh
    counts = np.bincount(bucket, minlength=n_buckets).reshape(nC, nW, nCh)
    counts_max = counts.max(axis=0)
    plan = make_plan(cfg, counts_max)

    order = np.argsort(bucket, kind="stable")
    sorted_bucket = bucket[order]
    seg_starts = np.searchsorted(sorted_bucket, np.arange(n_buckets))
    rank_sorted = np.arange(len(src)) - seg_starts[sorted_bucket]
    rank = np.empty_like(rank_sorted)
    rank[order] = rank_sorted

    slot_of = plan.bucket_slot_off[w_of, chunk_of] + rank

    deg_sq = np.sqrt(deg).astype(np.float32)

    from ml_dtypes import bfloat16

    prev = np.zeros((nW, nCh), dtype=np.int64)
    cum = np.cumsum(plan.tiles, axis=1)
    prev[:, 1:] = cum[:, :-1]

    def dstl_cols_for(mask):
        out = np.full((plan.dstl_tiles_total, WINDOW), -1.0, dtype=np.float32)
        sl = slot_of[mask]
        w = w_of[mask]
        c = chunk_of[mask]
        rel = sl - plan.bucket_slot_off[w, c]
        t_in_bucket = rel // WINDOW
        p = rel % WINDOW
        col = plan.w_tile_off[w] + prev[w, c] + t_in_bucket
        out[col, p] = dstl_rel[mask]
        return np.ascontiguousarray(out.T).astype(bfloat16)  # [128, tiles]

    in_maps = []
    for core in range(nC):
        xc = np.zeros((R, cfg.F), dtype=np.float32)
        take = min(N - core * R, R)
        xc[:take] = x[core * R: core * R + take]
        xT = np.ascontiguousarray(xc.T).astype(bfloat16)

        mask = core_of == core
        idx_stream = np.zeros(plan.total_slots, dtype=np.int16)
        idx_stream[slot_of[mask]] = idx_local[mask].astype(np.int16)
        idx_w = idx_stream.reshape(-1, 16).T
        idx_rep = np.tile(idx_w, (8, 1)).astype(np.int16)

        dstl_cols = dstl_cols_for(mask)

        dv = dinv_pad[core * R: (core + 1) * R].reshape(nW, WINDOW).T
        rd = np.ones((R,), dtype=np.float32)
        rd[:take] = deg_sq[core * R: core * R + take]
        rd = rd.reshape(nW, WINDOW).T

        iota = np.tile(np.arange(WINDOW, dtype=np.float32), (WINDOW, 1))
        iota_wide = np.tile(iota[:, None, :], (1, plan.T_max, 1))
        eye64 = np.eye(64, dtype=np.float32)
        eye128 = np.eye(128, dtype=np.float32)

        a09 = cfg.alpha / (1.0 - cfg.alpha)

        in_maps.append({
            "xT": xT,
            "W1": W1.astype(bfloat16),
            "b1": b1.reshape(cfg.H, 1).astype(np.float32),
            "W2": W2.astype(bfloat16),
            "b2": b2.reshape(cfg.C, 1).astype(np.float32),
            "iota_wide": iota_wide.astype(bfloat16),
            "eye64": eye64,
            "eye128": eye128.astype(bfloat16),
            "idxs": np.ascontiguousarray(idx_rep),
            "dstl": dstl_cols,
            "dinv_col": np.ascontiguousarray(dv),
            "h0w_col": np.ascontiguousarray(a09 * rd),
            "dinv09sq_col": np.ascontiguousarray((1.0 - cfg.alpha) * dv * dv),
            "rdinv_col": np.ascontiguousarray(rd),
        })
    return in_maps, plan


def build_kernel(cfg: Cfg, plan: Plan):
    nc = bacc.Bacc("TRN2", target_bir_lowering=False, debug=False,
                   num_devices=cfg.n_cores, num_swdge_queues=cfg.n_queues)
    _gq = [0]

    def emit_gather(gb_ap, src_ap, it_ap, ngc):
        o = 0
        while o < ngc:
            n = min(cfg.max_gather, ngc - o)
            dma_gather128(
                nc.gpsimd,
                gb_ap[:, o // 128:(o + n) // 128, :],
                src_ap,
                it_ap[:, o // 16:(o + n) // 16],
                n, n, cfg.C, ROWB,
                queue_num=_gq[0] % cfg.n_queues)
            _gq[0] += 1
            o += n

    R, nW, C, H, F = cfg.R, cfg.nW, cfg.C, cfg.H, cfg.F
    nCh, PR = cfg.n_chunks, cfg.part_rows

    xT_d = nc.dram_tensor("xT", [F, R], BF16, kind="ExternalInput")
    W1_d = nc.dram_tensor("W1", [F, H], BF16, kind="ExternalInput")
    b1_d = nc.dram_tensor("b1", [H, 1], F32, kind="ExternalInput")
    W2_d = nc.dram_tensor("W2", [H, C], BF16, kind="ExternalInput")
    b2_d = nc.dram_tensor("b2", [C, 1], F32, kind="ExternalInput")
    iota_d = nc.dram_tensor("iota_wide", [WINDOW, plan.T_max, WINDOW], BF16,
                            kind="ExternalInput")
    eye64_d = nc.dram_tensor("eye64", [64, 64], F32, kind="ExternalInput")
    eye128_d = nc.dram_tensor("eye128", [128, 128], BF16, kind="ExternalInput")
    idxs_d = nc.dram_tensor("idxs", [128, plan.idx_cols_total], I16,
                            kind="ExternalInput")
    dstl_d = nc.dram_tensor("dstl", [128, plan.dstl_tiles_total], BF16,
                            kind="ExternalInput")
    dinv_d = nc.dram_tensor("dinv_col", [WINDOW, nW], F32, kind="ExternalInput")
    h0w_d = nc.dram_tensor("h0w_col", [WINDOW, nW], F32, kind="ExternalInput")
    d9sq_d = nc.dram_tensor("dinv09sq_col", [WINDOW, nW], F32, kind="ExternalInput")
    rdinv_d = nc.dram_tensor("rdinv_col", [WINDOW, nW], F32, kind="ExternalInput")
    out_d = nc.dram_tensor("out", [R, C], F32, kind="ExternalOutput")

    groups = [list(range(cfg.n_cores))]

    with tile.TileContext(nc) as tc, ExitStack() as st:
        const = st.enter_context(tc.tile_pool(name="const", bufs=1))
        dram = st.enter_context(tc.tile_pool(name="dram", bufs=1, space="DRAM"))

        H_slice = dram.tile([R, ROWB], BF16)
        H_fulls = [[dram.tile([cfg.chunk_rows, ROWB], BF16, addr_space="Shared",
                              tag=f"hfull{k}_{p}", name=f"hfull{k}_{p}")
                    for p in range(cfg.n_parts)]
                   for k in range(cfg.K)]

        iota_sb = const.tile([WINDOW, plan.T_max, WINDOW], BF16, tag="iota")
        nc.sync.dma_start(iota_sb[:], iota_d[:])
        eye64_sb = const.tile([64, 64], F32, tag="eye64")
        nc.sync.dma_start(eye64_sb[:], eye64_d[:])
        eye128_sb = const.tile([128, 128], BF16, tag="eye128")
        nc.sync.dma_start(eye128_sb[:], eye128_d[:])
        dstl_sb = const.tile([128, plan.dstl_tiles_total], BF16, tag="dstl")
        nc.sync.dma_start(dstl_sb[:], dstl_d[:])
        dinv_sb = const.tile([WINDOW, nW], F32, tag="dinv")
        nc.sync.dma_start(dinv_sb[:], dinv_d[:])
        h0w_sb = const.tile([WINDOW, nW], F32, tag="h0w")
        nc.sync.dma_start(h0w_sb[:], h0w_d[:])
        d9sq_sb = const.tile([WINDOW, nW], F32, tag="d9sq")
        nc.sync.dma_start(d9sq_sb[:], d9sq_d[:])
        rdinv_sb = const.tile([WINDOW, nW], F32, tag="rdinv")
        nc.sync.dma_start(rdinv_sb[:], rdinv_d[:])
        h0pre_sb = const.tile([WINDOW, nW, C], BF16, tag="h0pre")

        W1t = []
        for kc in range(F // 128):
            t = const.tile([128, H], BF16, tag=f"w1_{kc}")
            nc.sync.dma_start(t[:], W1_d[kc * 128:(kc + 1) * 128, :])
            W1t.append(t)
        W2t = []
        for kc in range(H // 128):
            t = const.tile([128, C], BF16, tag=f"w2_{kc}")
            nc.sync.dma_start(t[:], W2_d[kc * 128:(kc + 1) * 128, :])
            W2t.append(t)
        b1c = []
        for hh in range(H // 128):
            t = const.tile([128, 1], F32, tag=f"b1_{hh}")
            nc.sync.dma_start(t[:], b1_d[hh * 128:(hh + 1) * 128, :])
            b1c.append(t)
        b2c = const.tile([C, 1], F32, tag="b2")
        nc.sync.dma_start(b2c[:], b2_d[:])

        # ---- phase 1: MLP -> h0pre (SBUF) and H'_0 -> H_slice (DRAM)
        with tc.tile_pool(name="mlp", bufs=3) as mp, \
             tc.tile_pool(name="mlp_ps", bufs=2, space="PSUM") as pp1, \
             tc.tile_pool(name="mlp_ps2", bufs=2, space="PSUM") as pp2, \
             tc.tile_pool(name="mlp_pst", bufs=2, space="PSUM") as ppt:
            r0 = 0
            while r0 < R:
                B = min(cfg.mlp_block, R - r0)
                xt = []
                for kc in range(F // 128):
                    t = mp.tile([128, cfg.mlp_block], BF16, tag=f"x_{kc}")
                    nc.sync.dma_start(t[:, :B], xT_d[kc * 128:(kc + 1) * 128, r0:r0 + B])
                    xt.append(t)
                h1 = []
                for half in range(H // 128):
                    ps = pp1.tile([128, cfg.mlp_block], F32, tag=f"ps1_{half}")
                    for kc in range(F // 128):
                        nc.tensor.matmul(
                            ps[:, :B],
                            W1t[kc][:, half * 128:(half + 1) * 128],
                            xt[kc][:, :B],
                            start=(kc == 0), stop=(kc == F // 128 - 1))
                    h = mp.tile([128, cfg.mlp_block], BF16, tag=f"h1_{half}")
                    nc.scalar.activation(h[:, :B], ps[:, :B], AF.Relu, bias=b1c[half][:])
                    h1.append(h)
                ps2 = pp2.tile([C, cfg.mlp_block], F32, tag="ps2")
                for kc in range(H // 128):
                    nc.tensor.matmul(ps2[:, :B], W2t[kc][:], h1[kc][:, :B],
                                     start=(kc == 0), stop=(kc == H // 128 - 1))
                hT = mp.tile([C, cfg.mlp_block], F32, tag="hT")
                nc.scalar.activation(hT[:, :B], ps2[:, :B], AF.Identity, bias=b2c[:])
                for j in range(B // WINDOW):
                    w = (r0 // WINDOW) + j
                    pst = ppt.tile([WINDOW, C], F32, tag="pst")
                    nc.tensor.transpose(pst[:], hT[:, j * WINDOW:(j + 1) * WINDOW], eye64_sb[:])
                    nc.vector.tensor_scalar_mul(h0pre_sb[:, w, :], pst[:],
                                                h0w_sb[:, w:w + 1])
                    hp = mp.tile([WINDOW, C], BF16, tag="hp")
                    nc.vector.tensor_scalar_mul(hp[:], pst[:], dinv_sb[:, w:w + 1])
                    nc.sync.dma_start(
                        H_slice[w * WINDOW:(w + 1) * WINDOW, :C], hp[:])
                r0 += B

        # ---- phase 2: K propagation hops
        hop_pools = {
            "idx": st.enter_context(tc.tile_pool(name="idx", bufs=3)),
            "gb": st.enter_context(tc.tile_pool(name="gb", bufs=2)),
            "S": st.enter_context(tc.tile_pool(name="S", bufs=cfg.G + 1)),
            "hw": st.enter_context(tc.tile_pool(name="hw", bufs=4)),
            "ps": st.enter_context(tc.tile_pool(name="ps", bufs=4, space="PSUM")),
        }

        def hop_body(HF):
            for p in range(cfg.n_parts):
                nc.gpsimd.collective_compute(
                    "AllGather", ALU.bypass, replica_groups=groups,
                    ins=[H_slice[p * PR:(p + 1) * PR, :].opt()],
                    outs=[HF[p].opt()])
            gmax_cols = int(plan.ng.sum(axis=1).max() // 16)
            for g in range(cfg.n_groups):
                gsum = int(plan.ng[g, :].sum())
                icol0 = int(plan.idx_col_off[g, 0])
                it_g = hop_pools["idx"].tile([128, gmax_cols], I16, tag="idxg")
                nc.sync.dma_start(it_g[:, :gsum // 16],
                                  idxs_d[:, icol0:icol0 + gsum // 16])
                gbufs = {}
                for c in range(nCh):
                    ngc = int(plan.ng[g, c])
                    if ngc == 0:
                        continue
                    ioff = int(plan.idx_col_off[g, c]) - icol0
                    gb = hop_pools["gb"].tile(
                        [128, int(plan.gbuf_tiles_max[c]), C], BF16, tag=f"gb{c}")
                    emit_gather(gb, HF[c][:, :C],
                                it_g[:, ioff:ioff + ngc // 16], ngc)
                    gbufs[c] = gb
                Ss = {}
                for w in cfg.group_windows(g):
                    T_w = int(plan.tiles[w, :].sum())
                    if T_w == 0:
                        continue
                    S = hop_pools["S"].tile([128, plan.T_max, 128], BF16, tag="S")
                    d0 = int(plan.w_tile_off[w])
                    bc = dstl_sb[:, d0:d0 + T_w, None].broadcast_to((128, T_w, 128))
                    nc.vector.tensor_tensor(S[:, :T_w, :], iota_sb[:, :T_w, :],
                                            bc, op=ALU.is_equal)
                    Ss[w] = S
                for w in cfg.group_windows(g):
                    T_w = int(plan.tiles[w, :].sum())
                    ps = hop_pools["ps"].tile([WINDOW, C], F32, tag="agg")
                    nc.tensor.matmul(ps[:], eye128_sb[:], h0pre_sb[:, w, :],
                                     start=True, stop=(T_w == 0))
                    done = 0
                    for c in range(nCh):
                        T = int(plan.tiles[w, c])
                        for t in range(T):
                            q = int(plan.gbuf_col_off[w, c]) + t
                            nc.tensor.matmul(
                                ps[:], Ss[w][:, done, :], gbufs[c][:, q, :],
                                start=False, stop=(done == T_w - 1))
                            done += 1
                    hp = hop_pools["hw"].tile([WINDOW, C], BF16, tag="hp2")
                    nc.vector.tensor_scalar_mul(hp[:], ps[:], d9sq_sb[:, w:w + 1])
                    nc.sync.dma_start(
                        H_slice[w * WINDOW:(w + 1) * WINDOW, :C], hp[:])

        for k in range(cfg.K):
            hop_body(H_fulls[k])

        # ---- phase 3: log_softmax
        with tc.tile_pool(name="sm", bufs=4) as smp, \
             tc.tile_pool(name="smc", bufs=4) as smc:
            for w in range(nW):
                hp = smp.tile([WINDOW, C], BF16, tag="hp3")
                nc.sync.dma_start(hp[:], H_slice[w * WINDOW:(w + 1) * WINDOW, :C])
                h = smp.tile([WINDOW, C], F32, tag="h3")
                nc.vector.tensor_scalar_mul(h[:], hp[:], rdinv_sb[:, w:w + 1])
                nm = smc.tile([WINDOW, 1], F32, tag="nm")
                nc.vector.tensor_reduce(nm[:], h[:], mybir.AxisListType.X,
                                        ALU.max, negate=True)
                e = smp.tile([WINDOW, C], F32, tag="e3")
                se = smc.tile([WINDOW, 1], F32, tag="se")
                nc.scalar.activation(e[:], h[:], AF.Exp, bias=nm[:], accum_out=se[:])
                ls = smc.tile([WINDOW, 1], F32, tag="ls")
                nc.scalar.activation(ls[:], se[:], AF.Ln)
                o = smp.tile([WINDOW, C], F32, tag="o3")
                nc.vector.tensor_scalar(o[:], h[:], nm[:], ls[:],
                                        op0=ALU.add, op1=ALU.subtract)
                nc.sync.dma_start(out_d[w * WINDOW:(w + 1) * WINDOW, :], o[:])

    nc.compile()
    return nc


# test-harness knobs (not used by the grading path, which calls kernel() only)
PROFILE = False
LAST_EXEC_NS = None
LAST_TRACE = None

_BUILD_CACHE: dict = {}


def _get_compiled(cfg: Cfg, plan: Plan):
    key = (cfg.N, cfg.E, cfg.K, cfg.G, plan.tiles.tobytes())
    hit = _BUILD_CACHE.get(key)
    if hit is None:
        hit = build_kernel(cfg, plan)
        _BUILD_CACHE.clear()
        _BUILD_CACHE[key] = hit
    return hit


def kernel(x, W1, b1, W2, b2, edge_index):
    """Full (unsharded) inputs in, full [N, 64] log-softmax output out."""
    from concourse.bass_utils import run_bass_kernel_spmd

    x = np.asarray(x, dtype=np.float32)
    W1 = np.asarray(W1, dtype=np.float32)
    b1 = np.asarray(b1, dtype=np.float32)
    W2 = np.asarray(W2, dtype=np.float32)
    b2 = np.asarray(b2, dtype=np.float32)
    edge_index = np.asarray(edge_index)

    N, F = x.shape
    H = W1.shape[1]
    C = W2.shape[1]
    E = edge_index.shape[1]
    cfg = Cfg(N=N, E=E, F=F, H=H, C=C, K=10, alpha=0.1, n_cores=8)

    in_maps, plan = host_prep(cfg, x, W1, b1, W2, b2, edge_index)
    nc = _get_compiled(cfg, plan)
    res = run_bass_kernel_spmd(nc, in_maps, list(range(cfg.n_cores)),
                               trace=PROFILE)
    if PROFILE:
        global LAST_EXEC_NS, LAST_TRACE
        LAST_EXEC_NS = res.exec_time_ns
        LAST_TRACE = (res.instructions_and_trace or (None, None))[1]
    out = np.concatenate([res.results[i]["out"] for i in range(cfg.n_cores)],
                         axis=0)[:N]
    return np.ascontiguousarray(out, dtype=np.float32)
